# revision 54
# baseline (speedup 1.0000x reference)
"""Trainium2 Bass kernel for AdapMultiSoftmaxAggrV2 (GNN segment-softmax aggregation).

Math (per reference):
    h = relu(x @ W + b)                      [N, 512]
    q = LayerNorm_128(h.reshape(N,4,128)) * querys
    a = segment_softmax(q, graph_idx)        (elementwise over Q,C)
    out[b] = sum_{n in b} a[n] * h[n]        [B, 512]

Identity used: out[b] = Num[b] / Den[b] with
    Num[b] = sum_{n in b} e[n] * h[n],  Den[b] = sum_{n in b} e[n],
    e = exp(q - SHIFT)   (softmax is shift-invariant)

Engine plan (CFG; HW-measured — the CoreSim Pool costs are ~5x optimistic,
so Pool only gets trivia):
  - PE: x@W in fp8-e4m3 DoubleRow (W pre-scaled by ws=4 to clear the e4m3
    denormal floor; ws divides out on the host). Den/Num segment sums are
    bf16 one-hot matmuls (S.T stationary, paired 64-partition halves).
  - ACT: PSUM->SBUF relu drains as 1024c pair instructions (2-bank PSUM
    tiles); exp as ONE wide ACT op per quad over the qprep'd chunks plus
    fused narrow exps (scale=rstd, bias=-mean*rstd-SHIFT APs) for
    CFG[fusedchunks] chunks; LN smalls (Ln+Exp for rstd) batched over
    quad pairs.
  - DVE: bn_stats (2 per chunk, (c,g)-interleaved so the even/odd split is
    the group split), qprep q=(h-mean)*rstd as 16 narrow (subtract,mult)
    tensor_scalars (fast-mode eligible), and the gmul g=e*h as one
    quad-wide 2048c bf16 tensor_tensor (2x mode).

Software pipeline: stages x(DMA) -> a(matmul+drain) -> s(stats) ->
m(rstd per pair) -> q(qprep) -> b(exp+gmul) -> c1(S DMA) -> c2(seg
matmuls), lag-scheduled per CFG[lags]/CFG[order].

Sharding: 8 cores, graph-parallel. Core i owns graphs [64i, 64(i+1)) and
their (contiguous, since graph_idx is sorted) node slice, zero-padded to a
common N_cap. Segment sums are one-hot matmuls (S.T stationary) accumulated
in PSUM across all node chunks. Host concatenates per-core [64, 512]
Num/Den and divides (plus the 1/ws fold).

Self-contained: only numpy + the concourse (Bass) runtime are imported.
"""

import numpy as np

NCORES = 8
QG, CH = 4, 128          # query groups, channels per group
MH = QG * CH             # 512
EPS_LN = 1e-5
EPS_SM = 1e-16

_COMPILED = {}  # (CK, with_bias, with_querys, G, cfg) -> nc

# Tunables. Changing these after a compile is cached has no effect on the
# cached program.
CFG = {
    "xw": "f8",      # f8 | bf16   x/W matmul dtype (f8 -> DoubleRow)
    "e": "bf16",     # f8 | bf16   e dtype (f8 -> DoubleRow paired Den)
    "g": "bf16",     # f8 | bf16   g dtype (f8 -> DoubleRow paired Num)
    "ws": 4.0,       # W pre-scale before fp8 quant (folded out on HOST: out /= ws)
    "shift": 3.5,    # e = exp(q - shift); cancels in Num/Den
    "psumpair": True,  # 2-bank PSUM tiles; drain 1024c per instr (2 chars in drain)
    "drain": "aa",    # per-pair (psumpair) or per-chunk engine: a=act d=dve s=split
    "qsplit": 0,      # how many of the 16 qprep ops ride DVE instead of Pool
    "fusedevery": 0,  # every Nth quad fully on the fused narrow-ACT-exp path
    "fusedchunks": 1,  # first k chunks of each quad fused (ACT/DVE balance)
    "fusetail": 0,    # last N quads fully fused (tail experiment; 0 = off)
    "dvetail": 0,     # last N quads drain on DVE (tail experiment; 0 = off)
    "fixedhalves": True,  # den at PE tile-pos (0,0), num at (0,64): no epilogue fold
    "gsplit": False,  # split gmul at the fused-chunk boundary (regresses; off)
    "gmul": "ttw",    # ttw = one quad-wide 2048c DVE tensor_tensor (2x bf16 mode);
                      # else per-chunk engine string (d=dve stt)
    "qprep": "dve",   # dve/pool = 16 narrow tensor_scalars + 1 quad-wide ACT exp;
                      # fused = 16 narrow ACT exps (scale/bias). Pool measured
                      # ~1.4us/op on real HW (5x the cost model) -> unusable.
}


def _build(CK, with_bias=False, with_querys=False, G=64, repeat=1):
    """Build + compile the per-core Bass program. CK = node chunks (x128).
    repeat>1 duplicates the whole pipeline (timing calibration only)."""
    import os
    ABLATE = set(os.environ.get("K_ABLATE", "").split(","))
    from contextlib import ExitStack

    import concourse.bass as bass
    import concourse.tile as tile
    from concourse import bacc, mybir

    f32 = mybir.dt.float32
    f32r = mybir.dt.float32r
    bf16 = mybir.dt.bfloat16
    f8 = mybir.dt.float8e4
    Alu = mybir.AluOpType
    AF = mybir.ActivationFunctionType
    DR = mybir.MatmulPerfMode.DoubleRow

    assert CK % 4 == 0, "CK must be a multiple of 4 (quad granularity)"
    assert G == 64, "col-tiled segment-sum halves assume 64 graphs per core"
    NQ = CK // 4
    SHIFT = float(CFG["shift"])
    WS = float(CFG["ws"])
    xw_f8 = CFG["xw"] == "f8"
    e_f8 = CFG["e"] == "f8"
    g_f8 = CFG["g"] == "f8"
    xdt = f8 if xw_f8 else bf16
    edt = f8 if e_f8 else bf16
    gdt = f8 if g_f8 else bf16
    ENG = {}  # filled after nc exists

    # Force the act-table chooser to the one set containing Relu+Exp+Ln+Copy
    # (natural_log_exp_and_others). The greedy chooser otherwise alternates
    # between exp_and_others and natural_log, reloading tables every quad
    # (~1.3us each). Empty out every other set; indices stay aligned.
    if not getattr(bacc, "_act_tables_patched", False):
        _orig_gat = bacc.get_activation_tables

        def _gat_only_nlexp(arch):
            tabs = _orig_gat(arch)
            keep = "natural_log_exp_and_others"
            if keep not in tabs:
                return tabs
            return {n: (fns if n == keep else set()) for n, fns in tabs.items()}

        bacc.get_activation_tables = _gat_only_nlexp
        bacc._act_tables_patched = True

    nc = bacc.Bacc("TRN2", target_bir_lowering=False, debug=False, num_devices=NCORES)
    ENG = {"p": nc.gpsimd, "d": nc.vector, "a": nc.scalar}

    def _bn_stats_raw(out_ap, in_ap):
        """bn_stats with a 3D interleaved input AP: stream order (c, g) with
        g innermost of size 2, so the HW's even/odd split lands exactly on
        the two groups -> out[...,0:3]=(count,mean,M2) of group 2j,
        out[...,3:6] of group 2j+1. Bypasses the bass shape assert (which
        assumes segment semantics walrus rejects)."""
        nc.vector.add_instruction(
            mybir.InstBNStats(
                name=nc.get_next_instruction_name(),
                ins=[nc.vector.lower_ap(in_ap)],
                outs=[nc.vector.lower_ap(out_ap)],
            )
        )

    xp = nc.dram_tensor("xp", [CK // 4, 128, 4, 4, 128], xdt, kind="ExternalInput")
    wt = nc.dram_tensor("wt", [128, 4, 512], xdt, kind="ExternalInput")
    need_s8 = e_f8 or g_f8
    need_sb = (not e_f8) or (not g_f8)
    # S blocks pre-transposed on host to [quad, p, c, G] so the per-quad DMA
    # is one contiguous block (the "c p g -> p c g" DMA costs ~512 tiny
    # descriptors per quad and swamps the HWDGE queue).
    if need_s8:
        st8_in = nc.dram_tensor("st8", [CK // 4, 128, 4, G], f8, kind="ExternalInput")
    if need_sb:
        stb_in = nc.dram_tensor("stb", [CK // 4, 128, 4, G], bf16, kind="ExternalInput")
    out = nc.dram_tensor("out", [2, G, 512], f32, kind="ExternalOutput")
    if with_bias:
        bvec = nc.dram_tensor("bvec", [1, 512], f32r, kind="ExternalInput")
    if with_querys:
        qw = nc.dram_tensor("qw", [128, 512], bf16, kind="ExternalInput")

    with tile.TileContext(nc) as tc, ExitStack() as ctx:
        psum_share = bool(CFG.get("psumshare")) and e_f8 and not g_f8
        consts = ctx.enter_context(tc.tile_pool(name="consts", bufs=1))
        xpool = ctx.enter_context(tc.tile_pool(name="xp", bufs=CFG.get("xbufs", 4)))
        hps = ctx.enter_context(tc.tile_pool(name="hps", bufs=7 if psum_share else 6, space="PSUM"))
        accps = ctx.enter_context(tc.tile_pool(name="acc", bufs=1, space="PSUM"))
        hbf = ctx.enter_context(tc.tile_pool(name="hbf", bufs=8))
        stp = ctx.enter_context(tc.tile_pool(name="st", bufs=8))
        smalls = ctx.enter_context(tc.tile_pool(name="sm", bufs=10))
        ep = ctx.enter_context(tc.tile_pool(name="e", bufs=6))
        gp = ctx.enter_context(tc.tile_pool(name="g", bufs=6))
        sp = ctx.enter_context(tc.tile_pool(name="s", bufs=8))
        outp = ctx.enter_context(tc.tile_pool(name="outp", bufs=1))

        # ---- constants ----
        wt_sb = consts.tile([128, 4, 512], xdt)
        nc.sync.dma_start(wt_sb[:], wt.ap())
        epsb = consts.tile([128, 1], f32)
        nc.any.memset(epsb[:], EPS_LN)
        if with_bias:
            b_sb = consts.tile([1, 512], f32r)
            nc.sync.dma_start(b_sb[:], bvec.ap())
            ones_sb = consts.tile([1, 128], f32r)
            nc.any.memset(ones_sb[:], 1.0)
        if with_querys:
            qw_sb = consts.tile([128, 512], bf16)
            nc.sync.dma_start(qw_sb[:], qw.ap())
        mshift = consts.tile([128, 1], f32)
        nc.any.memset(mshift[:], -SHIFT)

        # ---- persistent PSUM accumulators ----
        if psum_share:
            # DR den only writes partitions 0:64 of a bank; bf16 num can
            # accumulate in partitions 64:128 of the SAME bank via
            # tile_position (0,64) -> frees one PSUM bank for hp.
            acc = accps.tile([128, 512], f32)
            den_ps, num_ps = acc, acc
        else:
            den_ps = accps.tile([128, 512], f32)
            num_ps = accps.tile([128, 512], f32)

        # Software pipeline over quads: A(qd) computes h + LN params;
        # B(qd-1) computes e + g; C(qd-2/3) DMAs S and runs the segment-sum
        # matmuls. The lag keeps each engine's in-order stream from stalling
        # on the cross-engine chain of the newest quad.
        state = {}

        def stage_x(uid):
            # issue the x DMA LAG_A quads ahead of the matmuls that read it:
            # the ~1.7us DGE init + transfer latency must not sit on the
            # per-quad critical path.
            rep, qd = divmod(uid, NQ)
            x4 = xpool.tile([128, 4, 4, 128], xdt, name=f"x4_{uid}", tag="x4")
            nc.sync.dma_start(x4[:], xp.ap()[qd])
            state[uid] = {"x4": x4}

        def stage_a(uid):
            rep, qd = divmod(uid, NQ)
            x4 = state[uid]["x4"]
            if CFG.get("psumpair"):
                # two chunks share one 2-bank PSUM tile so the drain can be a
                # single 1024c instruction per pair (one ACT access bubble).
                # (A 4-bank/2048c variant doesn't fit: 2 quads in flight need
                # 8 hp banks + 2 accumulator banks > the 8 PSUM banks.)
                hp2 = [hps.tile([128, 2, 512], f32, tag="hp2", bufs=3,
                                name=f"hp2{uid}_{j}")
                       for j in range(2)]
                hp = [hp2[0][:, 0], hp2[0][:, 1], hp2[1][:, 0], hp2[1][:, 1]]
            else:
                hp = [hps.tile([128, 512], f32, tag="hp", name=f"hp{uid}_{i}")
                      for i in range(4)]
            for cq in range(4):  # chunk-in-quad
                dst = hp[cq][:]
                if xw_f8:
                    for j in range(2):
                        nc.tensor.matmul(
                            dst,
                            x4[:, cq, 2 * j : 2 * j + 2, :],
                            wt_sb[:, 2 * j : 2 * j + 2, :],
                            start=(j == 0),
                            stop=(j == 1 and not with_bias),
                            perf_mode=DR,
                        )
                else:
                    for k in range(4):
                        nc.tensor.matmul(
                            dst,
                            x4[:, cq, k, :],
                            wt_sb[:, k, :],
                            start=(k == 0),
                            stop=(k == 3 and not with_bias),
                        )
                if with_bias:
                    nc.tensor.matmul(dst, ones_sb[:], b_sb[:], start=False, stop=True)

            h4 = hbf.tile([128, 4, 512], bf16, name=f"h4_{uid}", tag="h4")
            if CFG.get("psumpair"):
                # pair drains: one 1024c instruction per 2-bank PSUM tile.
                # CFG["drain"] here is 2 chars (one per pair): a=act d=dve
                # s=split (ACT [0:768], DVE [768:1024]). The last
                # CFG[dvetail] quads drain on DVE: ACT is the pipe-drain
                # bottleneck while DVE idles there.
                tail = uid >= NQ * repeat - CFG.get("dvetail", 0)
                for j in range(2):
                    ec = "d" if tail else CFG["drain"][j]
                    src = hp2[j][:].rearrange("p a b -> p (a b)")
                    dst = h4[:, 2 * j : 2 * j + 2, :].rearrange("p a b -> p (a b)")
                    if ec == "a":
                        nc.scalar.activation(dst, src, AF.Relu)
                    elif ec == "s":
                        nc.scalar.activation(dst[:, 0:768], src[:, 0:768], AF.Relu)
                        nc.vector.tensor_scalar(
                            dst[:, 768:1024], src[:, 768:1024], 0.0, None, op0=Alu.max
                        )
                    else:
                        nc.vector.tensor_scalar(dst, src, 0.0, None, op0=Alu.max)
            else:
                for cq in range(4):
                    ec = CFG["drain"][cq]
                    if ec == "a":
                        nc.scalar.activation(h4[:, cq, :], hp[cq][:], AF.Relu)
                    elif ec == "s":  # split: ACT takes [0:256], DVE [256:512]
                        nc.scalar.activation(h4[:, cq, 0:256], hp[cq][:, 0:256], AF.Relu)
                        nc.vector.tensor_scalar(
                            h4[:, cq, 256:512], hp[cq][:, 256:512], 0.0, None, op0=Alu.max
                        )
                    else:
                        ENG[ec].tensor_scalar(
                            h4[:, cq, :], hp[cq][:], 0.0, None, op0=Alu.max
                        )
            state[uid]["h4"] = h4

        def stage_s(uid):
            rep, qd = divmod(uid, NQ)
            h4 = state[uid]["h4"]
            if "nostats" in ABLATE:
                rstd = smalls.tile([128, 16], f32, tag="rstd", name=f"rstd_{uid}")
                nc.any.memset(rstd[:], 1.0)
                mean0 = smalls.tile([128, 16], f32, tag="mean0", name=f"mean0_{uid}")
                nc.any.memset(mean0[:], 0.0)
                state[uid].update({"rstd_ap": rstd[:], "mean_ap": mean0[:]})
                return
            # LN stats: 2 bn_stats per chunk over (c, g)-interleaved views;
            # the even/odd split = the two groups directly, no merge needed.
            # Stats of MB consecutive quads share one tile so the Ln/Exp
            # smalls run as [128, MB*16] ops (one ACT access bubble per
            # batch, not per quad).
            MB = CFG.get("mbatch", 2)
            half = uid % MB
            if half == 0:
                st8 = stp.tile([128, MB, 48], f32, name=f"st8_{uid}", tag="st8")
                state[uid]["st8"] = st8
            else:
                st8 = state[uid - half]["st8"]
            for cq in range(4):
                for j in range(2):  # groups (2j, 2j+1)
                    _bn_stats_raw(
                        st8[:, half, cq * 12 + j * 6 : cq * 12 + j * 6 + 6],
                        h4[:, cq, j * 256 : j * 256 + 256]
                        .rearrange("p (g c) -> p c g", g=2),
                    )

        def stage_m(uid):
            # rstd for a quad batch: runs at the batch's last uid (or tail).
            if "nostats" in ABLATE:
                return
            MB = CFG.get("mbatch", 2)
            last = uid == NQ * repeat - 1
            if uid % MB != MB - 1 and not last:
                return
            nh = uid % MB + 1
            lead = uid - nh + 1
            st8 = state[lead]["st8"]
            stv = st8[:, 0:nh, :].rearrange("p h (x s) -> p (h x) s", s=3)
            mean_v, m2_v = stv[:, :, 1], stv[:, :, 2]  # [128, nh*16]
            # rstd = exp(-0.5 * ln(M2/128 + eps)); the /128 rides Ln's scale,
            # eps rides its bias -> no separate variance op
            W = nh * 16
            lnv = smalls.tile([128, MB * 16], f32, tag="lnv", name=f"lnv_{uid}")
            nc.scalar.activation(lnv[:, 0:W], m2_v, AF.Ln, bias=epsb[:], scale=1.0 / CH)
            rstd = smalls.tile([128, MB * 16], f32, tag="rstd", name=f"rstd_{uid}")
            nc.scalar.activation(rstd[:, 0:W], lnv[:, 0:W], AF.Exp, scale=-0.5)
            for k in range(nh):
                u = lead + k
                state[u]["rstd_ap"] = rstd[:, k * 16 : k * 16 + 16]
                state[u]["mean_ap"] = stv[:, k * 16 : k * 16 + 16, 1]
            if (CFG["qprep"] == "fused" or CFG.get("fusedevery")
                    or CFG.get("fusedchunks")):
                # narrow ACT exps need scale=rstd, bias=-mean*rstd - SHIFT
                nmr2 = smalls.tile([128, MB * 16], f32, tag="nmr2", name=f"nmr2_{uid}")
                nc.vector.scalar_tensor_tensor(
                    nmr2[:, 0:W], mean_v, -1.0, rstd[:, 0:W],
                    op0=Alu.mult, op1=Alu.mult)
                nc.gpsimd.tensor_scalar(nmr2[:, 0:W], nmr2[:, 0:W], -SHIFT,
                                        None, op0=Alu.add)
                for k in range(nh):
                    state[lead + k]["nmr2_ap"] = nmr2[:, k * 16 : k * 16 + 16]

        def _is_fused(uid):
            if with_querys or "noexp" in ABLATE:
                return False
            if CFG["qprep"] == "fused":
                return True
            if uid >= NQ * repeat - CFG.get("fusetail", 0):
                return True  # short-chain the last quads: faster pipe drain
            fe = CFG.get("fusedevery", 0)
            return bool(fe) and uid % fe == 0

        def _fused_chunks(uid):
            # first k chunks of every quad take the fused narrow-ACT-exp path
            # (no qprep); the rest go qprep + one wide exp. Balances ACT/DVE
            # at 1/4-quad granularity; fc0every drops the fused chunk on
            # every Nth quad for a finer average split.
            if _is_fused(uid):
                return 4
            f0 = CFG.get("fc0every", 0)
            if f0 and uid % f0 == 0:
                return 0
            return CFG.get("fusedchunks", 0)

        def stage_q(uid):
            # qprep: 16 narrow q = (h - mean) * rstd on DVE; its own stage so
            # the smalls chain gets lead time.
            if _fused_chunks(uid) == 4:
                return
            if "noexp" in ABLATE:
                return
            s = state[uid]
            h4, rstd, mean = s["h4"], s["rstd_ap"], s["mean_ap"]
            qeng = nc.gpsimd if CFG["qprep"] == "pool" else nc.vector
            qsplit = CFG.get("qsplit", 0)  # this many of the 16 ops go to DVE
            q4 = gp.tile([128, 4, 512], bf16, name=f"q4_{uid}", tag="q4")
            for _rep in range(CFG.get("qrep", 1)):  # >1: timing probes only
                for cq in range(_fused_chunks(uid), 4):
                    for g in range(QG):
                        col = cq * 4 + g
                        sl = slice(g * 128, g * 128 + 128)
                        eng = nc.vector if col >= 16 - qsplit else qeng
                        eng.tensor_scalar(
                            q4[:, cq, sl], h4[:, cq, sl],
                            mean[:, col : col + 1], rstd[:, col : col + 1],
                            op0=Alu.subtract, op1=Alu.mult,
                        )
            if with_querys:
                for cq in range(4):
                    nc.vector.tensor_tensor(
                        q4[:, cq, :], q4[:, cq, :], qw_sb[:], op=Alu.mult
                    )
            s["q4"] = q4

        def stage_b(uid):
            rep, qd = divmod(uid, NQ)
            s = state[uid]
            h4 = s["h4"]
            e4 = ep.tile([128, 4, 512], edt, name=f"e4_{uid}", tag="e4")
            fk = _fused_chunks(uid)
            if "noexp" in ABLATE:
                pass
            else:
                # chunks [0:fk): narrow ACT exps with per-partition scale/bias
                # (each pays the ~185ns ACT SBUF-access bubble); chunks [fk:4):
                # ONE wide ACT exp over the qprep'd columns.
                if fk:
                    rstd, nmr2 = s["rstd_ap"], s["nmr2_ap"]
                    for cq in range(fk):
                        for g in range(QG):
                            col = cq * 4 + g
                            nc.scalar.activation(
                                e4[:, cq, g * 128 : g * 128 + 128],
                                h4[:, cq, g * 128 : g * 128 + 128],
                                AF.Exp,
                                bias=nmr2[:, col : col + 1],
                                scale=rstd[:, col : col + 1],
                            )
                if fk < 4:
                    q4 = s.pop("q4")
                    nc.scalar.activation(
                        e4[:, fk:4, :].rearrange("p c f -> p (c f)"),
                        q4[:, fk:4, :].rearrange("p c f -> p (c f)"),
                        AF.Exp, bias=mshift[:], scale=1.0,
                    )
            # g = e * h; W's fp8 pre-scale ws rides h and is divided out on
            # the host (out /= ws).
            g4 = gp.tile([128, 4, 512], gdt, name=f"g4_{uid}", tag="g4")
            if "nogmul" not in ABLATE:
                if CFG["gmul"] in ("ttw", "ttwp"):
                    # quad-wide tensor_tensor; all-bf16 SBUF packed operands
                    # hit the DVE 2x perf mode. gsplit: emit the fused
                    # chunks' slice separately so it can run while the wide
                    # exp of the remaining chunks is still on ACT.
                    geng = nc.gpsimd if CFG["gmul"] == "ttwp" else nc.vector
                    gk = fk if (CFG.get("gsplit") and 0 < fk < 4) else 0
                    for _rep in range(CFG.get("grep", 1)):  # >1: probes only
                        if gk:
                            geng.tensor_tensor(
                                g4[:, 0:gk, :].rearrange("p c f -> p (c f)"),
                                e4[:, 0:gk, :].rearrange("p c f -> p (c f)"),
                                h4[:, 0:gk, :].rearrange("p c f -> p (c f)"),
                                op=Alu.mult,
                            )
                        geng.tensor_tensor(
                            g4[:, gk:4, :].rearrange("p c f -> p (c f)"),
                            e4[:, gk:4, :].rearrange("p c f -> p (c f)"),
                            h4[:, gk:4, :].rearrange("p c f -> p (c f)"),
                            op=Alu.mult,
                        )
                else:
                    for cq in range(4):
                        ENG[CFG["gmul"][cq]].scalar_tensor_tensor(
                            g4[:, cq, :], h4[:, cq, :], 1.0, e4[:, cq, :],
                            op0=Alu.mult, op1=Alu.mult,
                        )
            s["g4"] = g4
            s["e4"] = e4

        def stage_c1(uid):
            rep, qd = divmod(uid, NQ)
            s = state[uid]
            if need_s8:
                sT8 = sp.tile([128, 4, G], f8, name=f"sT8_{uid}", tag="sT8")
                nc.sync.dma_start(sT8[:], st8_in.ap()[qd])
                s["sT8"] = sT8
            if need_sb:
                sTb = sp.tile([128, 4, G], bf16, name=f"sTb_{uid}", tag="sTb")
                nc.sync.dma_start(sTb[:], stb_in.ap()[qd])
                s["sTb"] = sTb

        def stage_c2(uid):
            rep, qd = divmod(uid, NQ)
            s = state.pop(uid)
            first, last = uid == 0, uid == NQ * repeat - 1
            if CFG.get("fixedhalves") and not e_f8 and not g_f8 and not psum_share:
                # den accumulates at tile position (0,0) -> partitions 0:64,
                # num at (0,64) -> partitions 64:128. Interleaved so the PE
                # stationary loads still overlap, and the epilogue needs no
                # half-fold: the results DMA straight out of PSUM.
                for cq in range(4):
                    nc.tensor.matmul(
                        den_ps[0:64, :],
                        s["sTb"][:, cq, :], s["e4"][:, cq, :],
                        start=(first and cq == 0), stop=(last and cq == 3),
                        tile_position=(0, 0),
                    )
                    if "nogmul" not in ABLATE:
                        nc.tensor.matmul(
                            num_ps[64:128, :],
                            s["sTb"][:, cq, :], s["g4"][:, cq, :],
                            start=(first and cq == 0), stop=(last and cq == 3),
                            tile_position=(0, 64),
                        )
                return
            # Den: DoubleRow over chunk pairs (f8) or per-chunk (bf16).
            if e_f8:
                # DoubleRow requires tile_position (0,0): both chunk pairs
                # accumulate into the single [64,512] half of den_ps.
                for j in range(2):  # chunk pairs (2j, 2j+1)
                    nc.tensor.matmul(
                        den_ps[0:64, :],
                        s["sT8"][:, 2 * j : 2 * j + 2, :],
                        s["e4"][:, 2 * j : 2 * j + 2, :],
                        start=(first and j == 0), stop=(last and j == 1),
                        perf_mode=DR,
                    )
            else:
                for cq in range(4):
                    hd = cq % 2
                    nc.tensor.matmul(
                        den_ps[64 * hd : 64 * hd + 64, :],
                        s["sTb"][:, cq, :], s["e4"][:, cq, :],
                        start=(first and cq < 2), stop=(last and cq >= 2),
                        tile_position=(0, 64 * hd),
                    )
            if "nogmul" in ABLATE:
                return
            # Num
            if g_f8:
                for j in range(2):
                    nc.tensor.matmul(
                        num_ps[0:64, :],
                        s["sT8"][:, 2 * j : 2 * j + 2, :],
                        s["g4"][:, 2 * j : 2 * j + 2, :],
                        start=(first and j == 0), stop=(last and j == 1),
                        perf_mode=DR,
                    )
            elif psum_share:
                for cq in range(4):  # single-half accumulation at 64:128
                    nc.tensor.matmul(
                        num_ps[64:128, :],
                        s["sTb"][:, cq, :], s["g4"][:, cq, :],
                        start=(first and cq == 0), stop=(last and cq == 3),
                        tile_position=(0, 64),
                    )
            else:
                for cq in range(4):
                    hn = 1 - cq % 2
                    nc.tensor.matmul(
                        num_ps[64 * hn : 64 * hn + 64, :],
                        s["sTb"][:, cq, :], s["g4"][:, cq, :],
                        start=(first and cq < 2), stop=(last and cq >= 2),
                        tile_position=(0, 64 * hn),
                    )

        STAGES = {"x": stage_x, "a": stage_a, "s": stage_s, "m": stage_m,
                  "q": stage_q, "b": stage_b, "1": stage_c1, "2": stage_c2}
        LAGS = dict(CFG.get("lags", {"x": 0, "a": 2, "s": 3, "m": 3, "q": 5,
                                     "b": 8, "1": 8, "2": 9}))
        order = CFG.get("order", "x2smqba1")
        NT = NQ * repeat
        maxlag = max(LAGS.values())
        for it in range(NT + maxlag):
            for st in order:
                lag = LAGS[st]
                if not (lag <= it < NT + lag):
                    continue
                if st == "2" and "nosmm" in ABLATE:
                    continue
                STAGES[st](it - lag)

        # ---- epilogue: fold halves, DMA out ----
        if "nosmm" in ABLATE:
            nc.any.memset(den_ps[:], 1.0); nc.any.memset(num_ps[:], 1.0)
        if CFG.get("fixedhalves") and not e_f8 and not g_f8 and not psum_share:
            # no fold needed: den lives in den_ps[0:64], num in num_ps[64:128].
            # DMA can't read PSUM, so one copy each — on different engines so
            # they run in parallel.
            num_s = outp.tile([G, 512], f32)
            den_s = outp.tile([G, 512], f32)
            nc.scalar.copy(num_s[:], num_ps[G : 2 * G, :])
            nc.vector.tensor_scalar(den_s[:], den_ps[0:G, :], 0.0, None, op0=Alu.add)
            nc.sync.dma_start(out.ap()[0], num_s[:])
            nc.sync.dma_start(out.ap()[1], den_s[:])
        else:
            num_s = outp.tile([G, 512], f32)
            den_s = outp.tile([G, 512], f32)
            if g_f8:  # DR path accumulated everything in the low half
                nc.scalar.copy(num_s[:], num_ps[0:G, :])
            elif psum_share:
                nc.scalar.copy(num_s[:], num_ps[G : 2 * G, :])
            else:
                num_t = outp.tile([G, 512], f32)
                nc.scalar.copy(num_t[:], num_ps[0:G, :])
                nc.vector.tensor_tensor(num_s[:], num_t[:], num_ps[G : 2 * G, :], op=Alu.add)
            if e_f8:
                nc.scalar.copy(den_s[:], den_ps[0:G, :])
            else:
                den_t = outp.tile([G, 512], f32)
                nc.scalar.copy(den_t[:], den_ps[0:G, :])
                nc.vector.tensor_tensor(den_s[:], den_t[:], den_ps[G : 2 * G, :], op=Alu.add)
            nc.sync.dma_start(out.ap()[0], num_s[:])
            nc.sync.dma_start(out.ap()[1], den_s[:])

    nc.compile()
    return nc


def _get_compiled(CK, with_bias, with_querys, G=64):
    import json
    key = (CK, with_bias, with_querys, G, json.dumps(CFG, sort_keys=True, default=str))
    if key not in _COMPILED:
        _COMPILED[key] = _build(CK, with_bias, with_querys, G)
    return _COMPILED[key]


def _pack_core(x_sh, gl_sh, CK, G):
    """Pack one core's node slice into the DRAM layouts the kernel expects."""
    import ml_dtypes
    n = x_sh.shape[0]
    ncap = CK * 128
    xq = np.zeros((ncap, MH), dtype=np.float32)
    xq[:n] = x_sh
    # xp[c, p, k*128 + nn] = x[c*128 + nn, k*128 + p]
    xq = xq.reshape(CK, 128, QG, CH).transpose(0, 3, 2, 1).reshape(CK, 128, 512)
    # quad-contiguous: [qd, p, (chunk-in-quad, k, n)] so each quad is one
    # fully sequential DMA read
    xq = np.ascontiguousarray(
        xq.reshape(CK // 4, 4, 128, 512).transpose(0, 2, 1, 3)
        .reshape(CK // 4, 128, 4, 4, 128)
    )
    xdt = ml_dtypes.float8_e4m3 if CFG["xw"] == "f8" else ml_dtypes.bfloat16
    xq = xq.astype(xdt)
    # one-hot S.T blocks: st[c, p, g] = (graph_of_node(c*128+p) == g)
    gi = np.full((ncap,), -1, dtype=np.int64)
    gi[:n] = gl_sh
    st = (gi.reshape(CK, 128, 1) == np.arange(G).reshape(1, 1, G))
    # pre-transpose to [quad, p, c, G] for a contiguous per-quad DMA
    st = np.ascontiguousarray(
        st.reshape(CK // 4, 4, 128, G).transpose(0, 2, 1, 3)
    )
    return xq, st


def kernel(x, W_nlin, b_nlin, querys, graph_idx, batch_size):
    import ml_dtypes
    from concourse.bass_utils import run_bass_kernel_spmd

    x = np.asarray(x, dtype=np.float32)
    W = np.asarray(W_nlin, dtype=np.float32)
    b = np.asarray(b_nlin, dtype=np.float32)
    qs = np.asarray(querys, dtype=np.float32)
    gidx = np.asarray(graph_idx).astype(np.int64)
    if np.any(np.diff(gidx) < 0):  # kernel assumes sorted (contiguous segments);
        order = np.argsort(gidx, kind="stable")  # per-graph sums are order-invariant
        gidx = gidx[order]
        x = x[order]
    B = int(batch_size)
    N = x.shape[0]
    assert B % NCORES == 0, f"batch_size {B} not divisible by {NCORES}"
    G = B // NCORES

    with_bias = bool(np.any(b != 0))
    with_querys = not bool(np.all(qs == 1.0))

    # node counts per graph -> per-core contiguous node ranges (graph-aligned)
    counts = np.bincount(gidx, minlength=B)
    bounds = np.zeros(B + 1, dtype=np.int64)
    np.cumsum(counts, out=bounds[1:])
    starts = bounds[np.arange(NCORES) * G]
    ends = bounds[np.minimum(np.arange(NCORES) * G + G, B)]
    max_nodes = int((ends - starts).max())
    CK = -(-max_nodes // 512) * 4  # chunks of 128, rounded up to quads
    CK = max(CK, 4)

    nc = _get_compiled(CK, with_bias, with_querys, G)

    xw_f8 = CFG["xw"] == "f8"
    xdt = ml_dtypes.float8_e4m3 if xw_f8 else ml_dtypes.bfloat16
    ws = CFG["ws"] if xw_f8 else 1.0
    # W packed as wt[p, k, f] = W[k*128 + p, f]; pre-scaled into fp8 range
    wt = np.ascontiguousarray((W * ws).reshape(QG, CH, MH).transpose(1, 0, 2)).astype(xdt)

    need_s8 = CFG["e"] == "f8" or CFG["g"] == "f8"
    need_sb = CFG["e"] != "f8" or CFG["g"] != "f8"

    in_maps = []
    for i in range(NCORES):
        s, e = int(starts[i]), int(ends[i])
        xp_i, st_i = _pack_core(x[s:e], gidx[s:e] - i * G, CK, G)
        m = {"xp": xp_i, "wt": wt}
        if need_s8:
            m["st8"] = np.ascontiguousarray(st_i.astype(ml_dtypes.float8_e4m3))
        if need_sb:
            m["stb"] = np.ascontiguousarray(st_i.astype(ml_dtypes.bfloat16))
        if with_bias:
            m["bvec"] = np.ascontiguousarray((b * ws).reshape(1, MH))
        if with_querys:
            m["qw"] = np.broadcast_to(
                qs.reshape(1, MH), (128, MH)
            ).astype(ml_dtypes.bfloat16).copy()
        in_maps.append(m)

    ws_eff = CFG["ws"] if CFG["xw"] == "f8" else 1.0
    for attempt in range(3):
        res = run_bass_kernel_spmd(nc, in_maps, core_ids=list(range(NCORES)))
        outs = []
        for i in range(NCORES):
            o = res.results[i]["out"]  # [2, G, 512]: Num, Den (both carry
            outs.append(o[0] / (o[1] + EPS_SM) / ws_eff)  # e^-shift; h carries ws)
        full = np.concatenate(outs, axis=0).astype(np.float32)
        # A wedged device can return garbage without raising; Den >= ~n*e^-7
        # per graph on any real input, so NaN/Inf here means a bad execution.
        if np.isfinite(full).all():
            return full
    return full

